# revision 23
# baseline (speedup 1.0000x reference)
"""GQA forward kernel for 8 Trainium2 NeuronCores.

Problem: B=2, T=2048, C=2048, 32 Q heads / 8 KV heads, head_dim=64, causal.

Sharding: 2-way data parallel over batch x 4-way tensor parallel over KV-head
pairs. Each core handles one batch element and 2 KV heads (8 Q heads), computes
its slice of Q/K/V projections, causal attention, and a partial output
projection (transposed). Host sums the 4 partials per batch and adds bo.

Design (v5):
  - All matmul inputs bf16 (host converts); PSUM accumulation stays f32.
  - Attention is a single software-pipelined stripe stream per q-slice:
    scores run one stripe ahead of AV (covering the exp latency), continuing
    across head-pair (j) boundaries; PE filler units (next slice's
    projections, previous slice's output projection) are interleaved between
    stripes so the PE never waits on the ACT-bound exp stream.
  - PSUM: tag "st" 2x [128,1024] 2-bank slots (score pairs + normalization
    broadcast), tag "aux" 2x 1-bank slots (projection/oproj sub-GEMMs,
    V-transposes), av_a/av_b accumulator banks. Separate tags have separate
    allocation FIFOs so filler GEMMs schedule concurrently with stripes.
  - Scores for the two KV heads go into one [128,1024] 2-bank slot -> single
    exp Activation per stripe; the two K=64 score matmuls row-pack in the PE
    array (tile_position (0,0)/(64,0) derived from base partitions).
  - Softmax denominators via ones-column appended to V (rows 64/129 of v_s);
    normalization = 2 DVE row copies + 1 DVE reciprocal + 1 PE broadcast
    matmul + 2 DVE muls per (qs,j) -- no gpsimd.
  - Diagonal-stripe matmuls restrict their moving range to the unmasked
    columns; exp runs on the full [128,1024] tile (masked cols are zeroed by
    the PSUM bank clear, exp(0)=1 junk is never read by the restricted AV).
  - DMA batching: one strided transfer per xt t-slice, per weight tensor,
    per opt q-slice store, and one packed bias tile (HWDGE ring is serial
    per transfer, so fewer/bigger transfers).
"""

import sys
import numpy as np

T = 2048
C = 2048
D = 64
NT = 512          # t/q slice width (matmul moving free dim)
TS = T // NT      # 4 slices
KT = C // 128     # 16 contraction tiles
QC = 4            # local q-col tiles of 128 (512 local q cols)

_CACHE = {}


def _ensure_path():
    for p in ("/opt/trn_rl_repo",):
        if p not in sys.path:
            sys.path.insert(0, p)


def _build(reps=1, sections=("p1", "attn", "oproj")):
    sections = tuple(sections)
    key = (reps, sections)
    if key in _CACHE:
        return _CACHE[key]
    _ensure_path()
    import concourse.mybir as mybir
    import concourse.bacc as bacc
    from concourse import tile
    from concourse.masks import make_identity
    from contextlib import ExitStack

    dt = mybir.dt
    f32 = dt.float32
    bf16 = dt.bfloat16
    AF = mybir.ActivationFunctionType

    nc = bacc.Bacc(None, target_bir_lowering=False)
    xt_d = nc.declare_dram_parameter("xt", (C, T), bf16, isOutput=False)
    wq_d = nc.declare_dram_parameter("wq", (C, 512), bf16, isOutput=False)
    wk_d = nc.declare_dram_parameter("wk", (C, 128), bf16, isOutput=False)
    wv_d = nc.declare_dram_parameter("wv", (C, 128), bf16, isOutput=False)
    wo_d = nc.declare_dram_parameter("wo", (512, C), bf16, isOutput=False)
    bias_d = nc.declare_dram_parameter("bias", (128, 6), f32, isOutput=False)
    tri_d = nc.declare_dram_parameter("tri", (128, 128), bf16, isOutput=False)
    opt_d = nc.declare_dram_parameter("opt", (C, T), bf16, isOutput=True)

    with tile.TileContext(nc) as tc, ExitStack() as ctx:
        constp = ctx.enter_context(tc.tile_pool(name="const", bufs=1))
        wp = ctx.enter_context(tc.tile_pool(name="w", bufs=1))
        pers = ctx.enter_context(tc.tile_pool(name="pers", bufs=1))
        otp = ctx.enter_context(tc.tile_pool(name="ot", bufs=2))
        xtp = ctx.enter_context(tc.tile_pool(name="xt", bufs=2))
        tmpp = ctx.enter_context(tc.tile_pool(name="tmp", bufs=2))
        ptp = ctx.enter_context(tc.tile_pool(name="pt", bufs=6))
        rcp = ctx.enter_context(tc.tile_pool(name="rc", bufs=2))
        osp = ctx.enter_context(tc.tile_pool(name="os", bufs=2))
        # PSUM: st(2 slots x 2 banks) + aux(2 slots x 1 bank) + av_a(1)
        # + av_b(1) = 8 banks.
        pp_st = ctx.enter_context(tc.tile_pool(name="pst", bufs=2, space="PSUM"))
        pp_av = ctx.enter_context(tc.tile_pool(name="pav", bufs=1, space="PSUM"))

        # ---- constants & weights (loaded once, outside the reps loop)
        bias_s = constp.tile([128, 6], f32, tag="bias", name="bias")
        bq_s = [bias_s[:, j:j + 1] for j in range(QC)]
        bk_s = bias_s[:, 4:5]
        bv_s = bias_s[:, 5:6]
        tri_s = constp.tile([128, 128], bf16, tag="tri", name="tri")
        ident = constp.tile([128, 128], bf16, tag="ident", name="ident")
        make_identity(nc, ident[:])
        ones_c = constp.tile([128, 1], bf16, tag="ones_c", name="ones_c")
        nc.vector.memset(ones_c[:], 1.0)


        wqt = wp.tile([128, KT * 512], bf16, tag="wqt", name="wqt")
        wkt = wp.tile([128, KT * 128], bf16, tag="wkt", name="wkt")
        wvt = wp.tile([128, KT * 128], bf16, tag="wvt", name="wvt")
        wot = wp.tile([128, 4 * T], bf16, tag="wot", name="wot")
        _loaded = set()

        def wq_l(kt, qc):
            return wqt[:, kt * 512 + qc * 128:kt * 512 + (qc + 1) * 128]

        def wk_l(kt):
            return wkt[:, kt * 128:(kt + 1) * 128]

        def wv_l(kt):
            return wvt[:, kt * 128:(kt + 1) * 128]

        def wo_l(h, ct):
            return wot[:, h * T + ct * 128:h * T + (ct + 1) * 128]

        def ensure_qkv_w():
            if "qkv" in _loaded:
                return
            _loaded.add("qkv")
            for q4 in range(4):
                k0 = q4 * 4
                nc.sync.dma_start(
                    wqt[:, k0 * 512:(k0 + 4) * 512].rearrange(
                        "p (k c) -> p k c", k=4
                    ),
                    wq_d[k0 * 128:(k0 + 4) * 128, :].rearrange(
                        "(k p) c -> p k c", p=128
                    ),
                )
            nc.sync.dma_start(
                wkt[:].rearrange("p (k c) -> p k c", k=KT),
                wk_d[:].rearrange("(k p) c -> p k c", p=128),
            )
            nc.sync.dma_start(
                wvt[:].rearrange("p (k c) -> p k c", k=KT),
                wv_d[:].rearrange("(k p) c -> p k c", p=128),
            )

        def ensure_wo():
            if "wo" in _loaded:
                return
            _loaded.add("wo")
            nc.sync.dma_start(
                wot[:].rearrange("p (h c) -> p h c", h=4),
                wo_d[:].rearrange("(h p) c -> p h c", p=128),
            )

        def ensure_consts():
            if "consts" in _loaded:
                return
            _loaded.add("consts")
            nc.sync.dma_start(bias_s[:], bias_d[:])
            nc.sync.dma_start(tri_s[:], tri_d[:])

        if reps != 1:
            ensure_qkv_w()
            ensure_consts()
            ensure_wo()

        qt_s = [pers.tile([128, T], bf16, tag=f"qt{j}", name=f"qt{j}") for j in range(QC)]
        kt_s = pers.tile([128, T], bf16, tag="kt", name="kt")
        v_s = [pers.tile([128, 130], bf16, tag=f"vs{k}", name=f"vs{k}") for k in range(KT)]

        def p1_dma(ts):
            """Strided loads for the 16 xt c-stripes of t-slice ts (4 chunks
            so the first sub-GEMM can start early; first chunk precedes the
            weight loads so the reps=1 cold start is shorter)."""
            xts = xtp.tile([128, KT * NT], bf16, tag="xt", name="xt")

            def chunk(q):
                k0 = q * 4
                nc.sync.dma_start(
                    xts[:, k0 * NT:(k0 + 4) * NT].rearrange(
                        "p (k c) -> p k c", k=4
                    ),
                    xt_d[k0 * 128:(k0 + 4) * 128, ts * NT:(ts + 1) * NT]
                    .rearrange("(k p) c -> p k c", p=128),
                )
            chunk(0)
            ensure_qkv_w()
            ensure_consts()
            for q in range(1, 4):
                chunk(q)
            return xts

        def p1_units(ts, xts):
            """Projection work for t-slice ts as (emit_fn, pe_us) fillers."""
            units = []

            def q_unit(qc, lo_kt, hi_kt, ps_box):
                def emit():
                    if ps_box[0] is None:
                        ps_box[0] = pp_st.tile(
                            [128, NT], f32, tag="aux", bufs=2, name=f"psq{qc}"
                        )
                    ps = ps_box[0]
                    for kt in range(lo_kt, hi_kt):
                        nc.tensor.matmul(
                            ps[:],
                            wq_l(kt, qc),
                            xts[:, kt * NT:(kt + 1) * NT],
                            start=(kt == 0),
                            stop=(kt == KT - 1),
                        )
                    if hi_kt == KT:
                        nc.vector.tensor_scalar_add(
                            qt_s[qc][:, ts * NT:(ts + 1) * NT], ps[:], bq_s[qc]
                        )
                return emit, (hi_kt - lo_kt) * 0.21

            for qc in range(QC):
                box = [None]
                units.append(q_unit(qc, 0, 8, box))
                units.append(q_unit(qc, 8, KT, box))

            def k_unit():
                ps_k = pp_av.tile([128, NT], f32, tag="av_a", name="av_a")
                for kt in range(KT):
                    nc.tensor.matmul(
                        ps_k[:], wk_l(kt), xts[:, kt * NT:(kt + 1) * NT],
                        start=(kt == 0), stop=(kt == KT - 1),
                    )
                nc.vector.tensor_scalar_add(
                    kt_s[:, ts * NT:(ts + 1) * NT], ps_k[:], bk_s
                )
            units.append((k_unit, 3.4))

            vtmp_box = [None]

            def v_unit():
                ps_v = pp_av.tile([128, NT], f32, tag="av_b", name="av_b")
                for kt in range(KT):
                    nc.tensor.matmul(
                        ps_v[:], wv_l(kt), xts[:, kt * NT:(kt + 1) * NT],
                        start=(kt == 0), stop=(kt == KT - 1),
                    )
                vtmp = tmpp.tile([128, NT], bf16, tag="vtmp", name="vtmp")
                nc.vector.tensor_scalar_add(vtmp[:], ps_v[:], bv_s)
                vtmp_box[0] = vtmp
            units.append((v_unit, 3.4))

            def vt_unit(r):
                def emit():
                    k4 = ts * 4 + r
                    vtmp = vtmp_box[0]
                    tp_ps = pp_st.tile(
                        [128, 128], bf16, tag="aux", bufs=2, name="vtp"
                    )
                    nc.tensor.transpose(
                        tp_ps[:], vtmp[:, r * 128:(r + 1) * 128], ident[:]
                    )
                    nc.vector.tensor_copy(v_s[k4][:, 0:64], tp_ps[:, 0:64])
                    nc.vector.tensor_copy(v_s[k4][:, 65:129], tp_ps[:, 64:128])
                    nc.vector.tensor_copy(v_s[k4][:, 64:65], ones_c[:])
                    nc.vector.tensor_copy(v_s[k4][:, 129:130], ones_c[:])
                return emit
            for r in range(4):
                units.append((vt_unit(r), 0.3))
            return units

        def oproj_units(qs, ots):
            """Output projection for q-slice qs as filler units; the 16
            c-blocks accumulate into one [128, KT*NT] tile, stored with a
            single strided DMA by the last unit."""
            units = []
            os_box = [None]

            def ct_unit(ct):
                def emit():
                    if os_box[0] is None:
                        os_box[0] = osp.tile(
                            [128, KT * NT], bf16, tag="os", name="os"
                        )
                    op_ps = pp_st.tile([128, NT], f32, tag="aux", bufs=2, name="op")
                    for h in range(4):
                        nc.tensor.matmul(
                            op_ps[:],
                            wo_l(h, ct),
                            ots[h][:],
                            start=(h == 0),
                            stop=(h == 3),
                        )
                    nc.vector.tensor_copy(
                        os_box[0][:, ct * NT:(ct + 1) * NT], op_ps[:]
                    )
                    if ct in (KT // 2 - 1, KT - 1):
                        half = KT // 2
                        h0 = 0 if ct < half else half
                        nc.sync.dma_start(
                            opt_d[h0 * 128:(h0 + half) * 128,
                                  qs * NT:(qs + 1) * NT]
                            .rearrange("(k p) c -> p k c", p=128),
                            os_box[0][:, h0 * NT:(h0 + half) * NT]
                            .rearrange("p (k c) -> p k c", k=half),
                        )
                return emit
            for ct in range(KT):
                units.append((ct_unit(ct), 0.9))
            return units

        def attn_slice(qs, fillers):
            """Attention for q-slice qs (needs p1 ts<=qs). Software-pipelined
            stripe stream across all 4 head-pairs; `fillers` are interleaved
            between stripes. Returns the normalized head outputs (ots)."""
            total_fill = sum(c for _, c in fillers)
            nkt = 4 * qs + 4
            n_stripes = nkt * QC
            fillers = list(fillers)
            filled = 0.0
            done_stripes = 0

            def maybe_fill(force=0):
                nonlocal filled
                target = total_fill * done_stripes / n_stripes
                n = 0
                while fillers and (filled < target or n < force):
                    emit, cost = fillers.pop(0)
                    emit()
                    filled += cost
                    n += 1

            avs = {}
            ots = []

            def emit_av(j, pt, kt, lo):
                if j not in avs:
                    avs[j] = (
                        pp_av.tile([128, NT], f32, tag="av_a", name="av_a"),
                        pp_av.tile([128, NT], f32, tag="av_b", name="av_b"),
                    )
                av_a, av_b = avs[j]
                nc.tensor.matmul(
                    av_a[0:65, lo:NT],
                    v_s[kt][:, 0:65],
                    pt[:, lo:NT],
                    start=(kt == 0),
                    stop=(kt == nkt - 1),
                )
                nc.tensor.matmul(
                    av_b[0:65, lo:NT],
                    v_s[kt][:, 65:130],
                    pt[:, NT + lo:2 * NT],
                    start=(kt == 0),
                    stop=(kt == nkt - 1),
                )

            def emit_norm(j):
                av_a, av_b = avs[j]
                rc2 = rcp.tile([1, 2 * NT], f32, tag="rc2", name="rc2")
                nc.vector.reciprocal(rc2[0:1, 0:NT], av_a[64:65, :])
                nc.vector.reciprocal(rc2[0:1, NT:2 * NT], av_b[64:65, :])
                bc = rcp.tile([64, 2 * NT], f32, tag="bc", name="bc")
                nc.gpsimd.partition_broadcast(bc[:, 0:NT], rc2[:, 0:NT])
                nc.gpsimd.partition_broadcast(bc[:, NT:2 * NT], rc2[:, NT:2 * NT])
                ot_j = otp.tile([128, NT], bf16, tag=f"ot{j}", name=f"ot{j}")
                nc.vector.tensor_mul(ot_j[0:64, :], av_a[0:64, :], bc[:, 0:NT])
                nc.vector.tensor_mul(
                    ot_j[64:128, :], av_b[0:64, :], bc[:, NT:2 * NT]
                )
                ots.append(ot_j)

            prev = None
            for j in range(QC):
                for kt in range(nkt):
                    r = kt - 4 * qs
                    lo = max(r, 0) * 128  # first unmasked q col in this slice
                    st_t = pp_st.tile([128, 2 * NT], f32, tag="st", name="st")
                    nc.tensor.matmul(
                        st_t[:, lo:NT],
                        kt_s[0:64, kt * 128:(kt + 1) * 128],
                        qt_s[j][0:64, qs * NT + lo:(qs + 1) * NT],
                        start=True,
                        stop=True,
                    )
                    nc.tensor.matmul(
                        st_t[:, NT + lo:2 * NT],
                        kt_s[64:128, kt * 128:(kt + 1) * 128],
                        qt_s[j][64:128, qs * NT + lo:(qs + 1) * NT],
                        start=True,
                        stop=True,
                    )
                    pt = ptp.tile([128, 2 * NT], bf16, tag="pt", name="pt")
                    if lo:
                        nc.scalar.activation(
                            pt[:].rearrange("p (s c) -> p s c", s=2)[:, :, lo:NT],
                            st_t[:].rearrange("p (s c) -> p s c", s=2)[:, :, lo:NT],
                            AF.Exp,
                            scale=0.125,
                        )
                    else:
                        nc.scalar.activation(pt[:], st_t[:], AF.Exp, scale=0.125)
                    if r >= 0:
                        # causal mask inside the diagonal 128x128 block
                        nc.vector.tensor_mul(
                            pt[:, lo:lo + 128], pt[:, lo:lo + 128], tri_s[:]
                        )
                        nc.vector.tensor_mul(
                            pt[:, NT + lo:NT + lo + 128],
                            pt[:, NT + lo:NT + lo + 128],
                            tri_s[:],
                        )
                    if prev is not None:
                        pj, ppt, pkt, plo = prev
                        emit_av(pj, ppt, pkt, plo)
                        boundary = pkt == nkt - 1
                        if boundary:
                            emit_norm(pj)
                        done_stripes += 1
                        maybe_fill(force=3 if boundary else 0)
                    prev = (j, pt, kt, lo)
            pj, ppt, pkt, plo = prev
            emit_av(pj, ppt, pkt, plo)
            emit_norm(pj)
            while fillers:
                emit, cost = fillers.pop(0)
                emit()
            return ots

        def body(_iv=None):
            do_oproj = "oproj" in sections
            # compact first projection slice; wo load queued after its DMAs
            xts = p1_dma(0)
            for emit, _ in p1_units(0, xts):
                emit()
            if do_oproj:
                ensure_wo()
            all_ots = []
            for ts in range(TS):
                fillers = []
                if ts >= 1 and do_oproj:
                    fillers += oproj_units(ts - 1, all_ots[ts - 1])
                if ts < TS - 1:
                    nxt = p1_dma(ts + 1)
                    fillers += p1_units(ts + 1, nxt)
                all_ots.append(attn_slice(ts, fillers))
            if do_oproj:
                for emit, _ in oproj_units(TS - 1, all_ots[TS - 1]):
                    emit()
            else:
                for j in range(QC):
                    os_t = osp.tile([128, NT], bf16, tag="os2", name="os2")
                    nc.vector.tensor_copy(os_t[:], all_ots[TS - 1][j][:])
                    nc.sync.dma_start(
                        opt_d[j * 128:(j + 1) * 128, 0:NT], os_t[:]
                    )

        if reps == 1:
            body()
        else:
            hints = (
                mybir.EngineType.PE,
                mybir.EngineType.DVE,
                mybir.EngineType.Activation,
            )
            with tc.For_i(0, reps, 1, hint_engines=hints) as _i:
                body(_i)

    nc.compile()
    _CACHE[key] = nc
    return nc


def _make_in_maps(inputs):
    x = np.asarray(inputs["x"], np.float32)
    Wq = np.asarray(inputs["Wq"], np.float32)
    bq = np.asarray(inputs["bq"], np.float32)
    Wk = np.asarray(inputs["Wk"], np.float32)
    bk = np.asarray(inputs["bk"], np.float32)
    Wv = np.asarray(inputs["Wv"], np.float32)
    bv = np.asarray(inputs["bv"], np.float32)
    Wo = np.asarray(inputs["Wo"], np.float32)

    try:
        import ml_dtypes

        bf = ml_dtypes.bfloat16

        def to_bf16(a):
            return np.ascontiguousarray(a.astype(bf))
    except ImportError:
        def to_bf16(a):
            # round-to-nearest-even fp32 -> bf16, stored as uint16
            u = np.ascontiguousarray(a, np.float32).view(np.uint32)
            rounded = (u + 0x7FFF + ((u >> 16) & 1)) >> 16
            return np.ascontiguousarray(rounded.astype(np.uint16))

    tri = np.triu(np.ones((128, 128), np.float32))
    in_maps = []
    for c in range(8):
        b, tp = c // 4, c % 4
        k0, k1 = 2 * tp, 2 * tp + 1
        qorder = np.concatenate(
            [
                np.r_[(4 * k + j) * D:(4 * k + j + 1) * D]
                for j in range(4)
                for k in (k0, k1)
            ]
        )
        kvorder = np.r_[k0 * D:(k0 + 1) * D, k1 * D:(k1 + 1) * D]
        bias = np.zeros((128, 6), np.float32)
        bias[:, 0:4] = bq[qorder].reshape(4, 128).T
        bias[:, 4] = bk[kvorder][0:128]
        bias[:, 5] = bv[kvorder][0:128]
        in_maps.append(
            {
                "xt": to_bf16(x[b].T),
                "wq": to_bf16(Wq[:, qorder]),
                "wk": to_bf16(Wk[:, kvorder]),
                "wv": to_bf16(Wv[:, kvorder]),
                "wo": to_bf16(Wo[qorder, :]),
                "bias": bias,
                "tri": to_bf16(tri),
            }
        )
    return in_maps


def _gather(results, bo):
    out = np.zeros((2, T, C), np.float32)
    for c in range(8):
        out[c // 4] += results[c]["opt"].astype(np.float32).T
    out += bo.astype(np.float32)
    return out


def kernel(**inputs):
    _ensure_path()
    from concourse.bass_utils import run_bass_kernel_spmd

    nc = _build(reps=1)
    in_maps = _make_in_maps(inputs)
    res = run_bass_kernel_spmd(nc, in_maps, list(range(8)))
    return _gather(res.results, np.asarray(inputs["bo"], np.float32))


def run_timed(inputs, reps, n_calls=3, sections=("p1", "attn", "oproj")):
    """Wall-clock the SPMD call at a given in-kernel rep count; returns
    (best_wall_seconds, outputs). Kernel time per rep is isolated by
    differencing two rep counts (data transfer is identical)."""
    import time

    _ensure_path()
    from concourse.bass_utils import run_bass_kernel_spmd

    nc = _build(reps=reps, sections=sections)
    in_maps = _make_in_maps(inputs)
    best = None
    res = None
    for _ in range(n_calls):
        t0 = time.time()
        res = run_bass_kernel_spmd(nc, in_maps, list(range(8)))
        dtm = time.time() - t0
        best = dtm if best is None else min(best, dtm)
    return best, _gather(res.results, np.asarray(inputs["bo"], np.float32))


# revision 27
# speedup vs baseline: 10.6447x; 10.6447x over previous
"""GQA forward kernel for 8 Trainium2 NeuronCores.

Problem: B=2, T=2048, C=2048, 32 Q heads / 8 KV heads, head_dim=64, causal.

Sharding: 2-way data parallel over batch x 4-way tensor parallel over KV-head
pairs. Each core handles one batch element and 2 KV heads (8 Q heads), computes
its slice of Q/K/V projections, causal attention, and a partial output
projection (transposed). Host sums the 4 partials per batch and adds bo.

Design (v5):
  - All matmul inputs bf16 (host converts); PSUM accumulation stays f32.
  - Attention is a single software-pipelined stripe stream per q-slice:
    scores run one stripe ahead of AV (covering the exp latency), continuing
    across head-pair (j) boundaries; PE filler units (next slice's
    projections, previous slice's output projection) are interleaved between
    stripes so the PE never waits on the ACT-bound exp stream.
  - PSUM: tag "st" 2x [128,1024] 2-bank slots (score pairs + normalization
    broadcast), tag "aux" 2x 1-bank slots (projection/oproj sub-GEMMs,
    V-transposes), av_a/av_b accumulator banks. Separate tags have separate
    allocation FIFOs so filler GEMMs schedule concurrently with stripes.
  - Scores for the two KV heads go into one [128,1024] 2-bank slot -> single
    exp Activation per stripe; the two K=64 score matmuls row-pack in the PE
    array (tile_position (0,0)/(64,0) derived from base partitions).
  - Softmax denominators via ones-column appended to V (rows 64/129 of v_s);
    normalization = 2 DVE row copies + 1 DVE reciprocal + 1 PE broadcast
    matmul + 2 DVE muls per (qs,j) -- no gpsimd.
  - Diagonal-stripe matmuls restrict their moving range to the unmasked
    columns; exp runs on the full [128,1024] tile (masked cols are zeroed by
    the PSUM bank clear, exp(0)=1 junk is never read by the restricted AV).
  - DMA batching: one strided transfer per xt t-slice, per weight tensor,
    per opt q-slice store, and one packed bias tile (HWDGE ring is serial
    per transfer, so fewer/bigger transfers).
"""

import sys
import numpy as np

T = 2048
C = 2048
D = 64
NT = 512          # t/q slice width (matmul moving free dim)
TS = T // NT      # 4 slices
KT = C // 128     # 16 contraction tiles
QC = 4            # local q-col tiles of 128 (512 local q cols)

_CACHE = {}


def _ensure_path():
    for p in ("/opt/trn_rl_repo",):
        if p not in sys.path:
            sys.path.insert(0, p)


def _build(reps=1, sections=("p1", "attn", "oproj")):
    sections = tuple(sections)
    key = (reps, sections)
    if key in _CACHE:
        return _CACHE[key]
    _ensure_path()
    import concourse.mybir as mybir
    import concourse.bacc as bacc
    from concourse import tile
    from concourse.masks import make_identity
    from contextlib import ExitStack

    dt = mybir.dt
    f32 = dt.float32
    bf16 = dt.bfloat16
    AF = mybir.ActivationFunctionType

    nc = bacc.Bacc(None, target_bir_lowering=False)
    xt_d = nc.declare_dram_parameter("xt", (C, T), bf16, isOutput=False)
    wq_d = nc.declare_dram_parameter("wq", (C, 512), bf16, isOutput=False)
    wk_d = nc.declare_dram_parameter("wk", (C, 128), bf16, isOutput=False)
    wv_d = nc.declare_dram_parameter("wv", (C, 128), bf16, isOutput=False)
    wo_d = nc.declare_dram_parameter("wo", (512, C), bf16, isOutput=False)
    bias_d = nc.declare_dram_parameter("bias", (128, 6), f32, isOutput=False)
    tri_d = nc.declare_dram_parameter("tri", (128, 128), bf16, isOutput=False)
    opt_d = nc.declare_dram_parameter("opt", (C, T), bf16, isOutput=True)

    with tile.TileContext(nc) as tc, ExitStack() as ctx:
        constp = ctx.enter_context(tc.tile_pool(name="const", bufs=1))
        wp = ctx.enter_context(tc.tile_pool(name="w", bufs=1))
        pers = ctx.enter_context(tc.tile_pool(name="pers", bufs=1))
        otp = ctx.enter_context(tc.tile_pool(name="ot", bufs=2))
        xtp = ctx.enter_context(tc.tile_pool(name="xt", bufs=2))
        tmpp = ctx.enter_context(tc.tile_pool(name="tmp", bufs=2))
        ptp = ctx.enter_context(tc.tile_pool(name="pt", bufs=6))
        rcp = ctx.enter_context(tc.tile_pool(name="rc", bufs=2))
        osp = ctx.enter_context(tc.tile_pool(name="os", bufs=2))
        # PSUM: st(2 slots x 2 banks) + aux(2 slots x 1 bank) + av_a(1)
        # + av_b(1) = 8 banks.
        pp_st = ctx.enter_context(tc.tile_pool(name="pst", bufs=2, space="PSUM"))
        pp_av = ctx.enter_context(tc.tile_pool(name="pav", bufs=1, space="PSUM"))

        # ---- constants & weights (loaded once, outside the reps loop)
        bias_s = constp.tile([128, 6], f32, tag="bias", name="bias")
        bq_s = [bias_s[:, j:j + 1] for j in range(QC)]
        bk_s = bias_s[:, 4:5]
        bv_s = bias_s[:, 5:6]
        tri_s = constp.tile([128, 128], bf16, tag="tri", name="tri")
        ident = constp.tile([128, 128], bf16, tag="ident", name="ident")
        make_identity(nc, ident[:])
        ones_c = constp.tile([128, 1], bf16, tag="ones_c", name="ones_c")
        nc.vector.memset(ones_c[:], 1.0)


        wqt = wp.tile([128, KT * 512], bf16, tag="wqt", name="wqt")
        wkt = wp.tile([128, KT * 128], bf16, tag="wkt", name="wkt")
        wvt = wp.tile([128, KT * 128], bf16, tag="wvt", name="wvt")
        wot = wp.tile([128, 4 * T], bf16, tag="wot", name="wot")
        _loaded = set()

        def wq_l(kt, qc):
            return wqt[:, kt * 512 + qc * 128:kt * 512 + (qc + 1) * 128]

        def wk_l(kt):
            return wkt[:, kt * 128:(kt + 1) * 128]

        def wv_l(kt):
            return wvt[:, kt * 128:(kt + 1) * 128]

        def wo_l(h, ct):
            return wot[:, h * T + ct * 128:h * T + (ct + 1) * 128]

        def ensure_qkv_w():
            if "qkv" in _loaded:
                return
            _loaded.add("qkv")
            for q4 in range(4):
                k0 = q4 * 4
                nc.sync.dma_start(
                    wqt[:, k0 * 512:(k0 + 4) * 512].rearrange(
                        "p (k c) -> p k c", k=4
                    ),
                    wq_d[k0 * 128:(k0 + 4) * 128, :].rearrange(
                        "(k p) c -> p k c", p=128
                    ),
                )
            nc.sync.dma_start(
                wkt[:].rearrange("p (k c) -> p k c", k=KT),
                wk_d[:].rearrange("(k p) c -> p k c", p=128),
            )
            nc.sync.dma_start(
                wvt[:].rearrange("p (k c) -> p k c", k=KT),
                wv_d[:].rearrange("(k p) c -> p k c", p=128),
            )

        def ensure_wo():
            if "wo" in _loaded:
                return
            _loaded.add("wo")
            nc.sync.dma_start(
                wot[:].rearrange("p (h c) -> p h c", h=4),
                wo_d[:].rearrange("(h p) c -> p h c", p=128),
            )

        def ensure_consts():
            if "consts" in _loaded:
                return
            _loaded.add("consts")
            nc.sync.dma_start(bias_s[:], bias_d[:])
            nc.sync.dma_start(tri_s[:], tri_d[:])

        if reps != 1:
            ensure_qkv_w()
            ensure_consts()
            ensure_wo()

        qt_s = [pers.tile([128, T], bf16, tag=f"qt{j}", name=f"qt{j}") for j in range(QC)]
        kt_s = pers.tile([128, T], bf16, tag="kt", name="kt")
        v_s = [pers.tile([128, 130], bf16, tag=f"vs{k}", name=f"vs{k}") for k in range(KT)]

        def p1_dma(ts):
            """Strided loads for the 16 xt c-stripes of t-slice ts (4 chunks
            so the first sub-GEMM can start early; first chunk precedes the
            weight loads so the reps=1 cold start is shorter)."""
            xts = xtp.tile([128, KT * NT], bf16, tag="xt", name="xt")

            def chunk(q):
                k0 = q * 4
                nc.sync.dma_start(
                    xts[:, k0 * NT:(k0 + 4) * NT].rearrange(
                        "p (k c) -> p k c", k=4
                    ),
                    xt_d[k0 * 128:(k0 + 4) * 128, ts * NT:(ts + 1) * NT]
                    .rearrange("(k p) c -> p k c", p=128),
                )
            chunk(0)
            ensure_qkv_w()
            ensure_consts()
            for q in range(1, 4):
                chunk(q)
            return xts

        def p1_units(ts, xts):
            """Projection work for t-slice ts as (emit_fn, pe_us) fillers."""
            units = []

            def q_unit(qc, lo_kt, hi_kt, ps_box):
                def emit():
                    if ps_box[0] is None:
                        ps_box[0] = pp_st.tile(
                            [128, NT], f32, tag="aux", bufs=2, name=f"psq{qc}"
                        )
                    ps = ps_box[0]
                    for kt in range(lo_kt, hi_kt):
                        nc.tensor.matmul(
                            ps[:],
                            wq_l(kt, qc),
                            xts[:, kt * NT:(kt + 1) * NT],
                            start=(kt == 0),
                            stop=(kt == KT - 1),
                        )
                    if hi_kt == KT:
                        nc.vector.tensor_scalar_add(
                            qt_s[qc][:, ts * NT:(ts + 1) * NT], ps[:], bq_s[qc]
                        )
                return emit, (hi_kt - lo_kt) * 0.21

            for qc in range(QC):
                box = [None]
                units.append(q_unit(qc, 0, 8, box))
                units.append(q_unit(qc, 8, KT, box))

            def k_unit():
                ps_k = pp_av.tile([128, NT], f32, tag="av_a", name="av_a")
                for kt in range(KT):
                    nc.tensor.matmul(
                        ps_k[:], wk_l(kt), xts[:, kt * NT:(kt + 1) * NT],
                        start=(kt == 0), stop=(kt == KT - 1),
                    )
                nc.vector.tensor_scalar_add(
                    kt_s[:, ts * NT:(ts + 1) * NT], ps_k[:], bk_s
                )
            units.append((k_unit, 3.4))

            vtmp_box = [None]

            def v_unit():
                ps_v = pp_av.tile([128, NT], f32, tag="av_b", name="av_b")
                for kt in range(KT):
                    nc.tensor.matmul(
                        ps_v[:], wv_l(kt), xts[:, kt * NT:(kt + 1) * NT],
                        start=(kt == 0), stop=(kt == KT - 1),
                    )
                vtmp = tmpp.tile([128, NT], bf16, tag="vtmp", name="vtmp")
                nc.vector.tensor_scalar_add(vtmp[:], ps_v[:], bv_s)
                vtmp_box[0] = vtmp
            units.append((v_unit, 3.4))

            def vt_unit(r):
                def emit():
                    k4 = ts * 4 + r
                    vtmp = vtmp_box[0]
                    tp_ps = pp_st.tile(
                        [128, 128], bf16, tag="aux", bufs=2, name="vtp"
                    )
                    nc.tensor.transpose(
                        tp_ps[:], vtmp[:, r * 128:(r + 1) * 128], ident[:]
                    )
                    nc.vector.tensor_copy(v_s[k4][:, 0:64], tp_ps[:, 0:64])
                    nc.vector.tensor_copy(v_s[k4][:, 65:129], tp_ps[:, 64:128])
                    nc.vector.tensor_copy(v_s[k4][:, 64:65], ones_c[:])
                    nc.vector.tensor_copy(v_s[k4][:, 129:130], ones_c[:])
                return emit
            for r in range(4):
                units.append((vt_unit(r), 0.3))
            return units

        def oproj_units(qs, ots):
            """Output projection for q-slice qs as filler units; the 16
            c-blocks accumulate into one [128, KT*NT] tile, stored with a
            single strided DMA by the last unit."""
            units = []
            os_box = [None]

            def ct_unit(ct):
                def emit():
                    if os_box[0] is None:
                        os_box[0] = osp.tile(
                            [128, KT * NT], bf16, tag="os", name="os"
                        )
                    op_ps = pp_st.tile([128, NT], f32, tag="aux", bufs=2, name="op")
                    for h in range(4):
                        nc.tensor.matmul(
                            op_ps[:],
                            wo_l(h, ct),
                            ots[h][:],
                            start=(h == 0),
                            stop=(h == 3),
                        )
                    nc.vector.tensor_copy(
                        os_box[0][:, ct * NT:(ct + 1) * NT], op_ps[:]
                    )
                    if ct in (KT // 2 - 1, KT - 1):
                        half = KT // 2
                        h0 = 0 if ct < half else half
                        nc.sync.dma_start(
                            opt_d[h0 * 128:(h0 + half) * 128,
                                  qs * NT:(qs + 1) * NT]
                            .rearrange("(k p) c -> p k c", p=128),
                            os_box[0][:, h0 * NT:(h0 + half) * NT]
                            .rearrange("p (k c) -> p k c", k=half),
                        )
                return emit
            for ct in range(KT):
                units.append((ct_unit(ct), 0.9))
            return units

        def attn_slice(qs, fillers):
            """Attention for q-slice qs (needs p1 ts<=qs). Software-pipelined
            stripe stream across all 4 head-pairs; `fillers` are interleaved
            between stripes. Returns the normalized head outputs (ots)."""
            total_fill = sum(c for _, c in fillers)
            nkt = 4 * qs + 4
            n_stripes = nkt * QC
            fillers = list(fillers)
            filled = 0.0
            done_stripes = 0

            def maybe_fill(force=0):
                nonlocal filled
                target = total_fill * done_stripes / n_stripes
                n = 0
                while fillers and (filled < target or n < force):
                    emit, cost = fillers.pop(0)
                    emit()
                    filled += cost
                    n += 1

            avs = {}
            ots = []

            def emit_av(j, pt, kt, lo):
                if j not in avs:
                    avs[j] = (
                        pp_av.tile([128, NT], f32, tag="av_a", name="av_a"),
                        pp_av.tile([128, NT], f32, tag="av_b", name="av_b"),
                    )
                av_a, av_b = avs[j]
                nc.tensor.matmul(
                    av_a[0:65, lo:NT],
                    v_s[kt][:, 0:65],
                    pt[:, lo:NT],
                    start=(kt == 0),
                    stop=(kt == nkt - 1),
                )
                nc.tensor.matmul(
                    av_b[0:65, lo:NT],
                    v_s[kt][:, 65:130],
                    pt[:, NT + lo:2 * NT],
                    start=(kt == 0),
                    stop=(kt == nkt - 1),
                )

            def emit_norm(j):
                av_a, av_b = avs[j]
                rc2 = rcp.tile([1, 2 * NT], f32, tag="rc2", name="rc2")
                nc.vector.reciprocal(rc2[0:1, 0:NT], av_a[64:65, :])
                nc.vector.reciprocal(rc2[0:1, NT:2 * NT], av_b[64:65, :])
                bc = rcp.tile([64, 2 * NT], f32, tag="bc", name="bc")
                nc.gpsimd.partition_broadcast(bc[:, 0:NT], rc2[:, 0:NT])
                nc.gpsimd.partition_broadcast(bc[:, NT:2 * NT], rc2[:, NT:2 * NT])
                ot_j = otp.tile([128, NT], bf16, tag=f"ot{j}", name=f"ot{j}")
                nc.vector.tensor_mul(ot_j[0:64, :], av_a[0:64, :], bc[:, 0:NT])
                nc.vector.tensor_mul(
                    ot_j[64:128, :], av_b[0:64, :], bc[:, NT:2 * NT]
                )
                ots.append(ot_j)

            prev = None
            for j in range(QC):
                for kt in range(nkt):
                    r = kt - 4 * qs
                    lo = max(r, 0) * 128  # first unmasked q col in this slice
                    st_t = pp_st.tile([128, 2 * NT], f32, tag="st", name="st")
                    nc.tensor.matmul(
                        st_t[:, lo:NT],
                        kt_s[0:64, kt * 128:(kt + 1) * 128],
                        qt_s[j][0:64, qs * NT + lo:(qs + 1) * NT],
                        start=True,
                        stop=True,
                    )
                    nc.tensor.matmul(
                        st_t[:, NT + lo:2 * NT],
                        kt_s[64:128, kt * 128:(kt + 1) * 128],
                        qt_s[j][64:128, qs * NT + lo:(qs + 1) * NT],
                        start=True,
                        stop=True,
                    )
                    pt = ptp.tile([128, 2 * NT], bf16, tag="pt", name="pt")
                    if lo:
                        nc.scalar.activation(
                            pt[:].rearrange("p (s c) -> p s c", s=2)[:, :, lo:NT],
                            st_t[:].rearrange("p (s c) -> p s c", s=2)[:, :, lo:NT],
                            AF.Exp,
                            scale=0.125,
                        )
                    else:
                        nc.scalar.activation(pt[:], st_t[:], AF.Exp, scale=0.125)
                    if r >= 0:
                        # causal mask inside the diagonal 128x128 block
                        nc.vector.tensor_mul(
                            pt[:, lo:lo + 128], pt[:, lo:lo + 128], tri_s[:]
                        )
                        nc.vector.tensor_mul(
                            pt[:, NT + lo:NT + lo + 128],
                            pt[:, NT + lo:NT + lo + 128],
                            tri_s[:],
                        )
                    if prev is not None:
                        pj, ppt, pkt, plo = prev
                        emit_av(pj, ppt, pkt, plo)
                        boundary = pkt == nkt - 1
                        if boundary:
                            emit_norm(pj)
                        done_stripes += 1
                        maybe_fill(force=3 if boundary else 0)
                    prev = (j, pt, kt, lo)
            pj, ppt, pkt, plo = prev
            emit_av(pj, ppt, pkt, plo)
            emit_norm(pj)
            while fillers:
                emit, cost = fillers.pop(0)
                emit()
            return ots

        def body(_iv=None):
            do_oproj = "oproj" in sections
            # compact first projection slice; wo load queued after its DMAs
            xts = p1_dma(0)
            for emit, _ in p1_units(0, xts):
                emit()
            if do_oproj:
                ensure_wo()
            all_ots = []
            for ts in range(TS):
                fillers = []
                if ts >= 1 and do_oproj:
                    fillers += oproj_units(ts - 1, all_ots[ts - 1])
                if ts < TS - 1:
                    nxt = p1_dma(ts + 1)
                    fillers += p1_units(ts + 1, nxt)
                all_ots.append(attn_slice(ts, fillers))
            if do_oproj:
                for emit, _ in oproj_units(TS - 1, all_ots[TS - 1]):
                    emit()
            else:
                for j in range(QC):
                    os_t = osp.tile([128, NT], bf16, tag="os2", name="os2")
                    nc.vector.tensor_copy(os_t[:], all_ots[TS - 1][j][:])
                    nc.sync.dma_start(
                        opt_d[j * 128:(j + 1) * 128, 0:NT], os_t[:]
                    )

        if reps == 1:
            body()
        else:
            hints = (
                mybir.EngineType.PE,
                mybir.EngineType.DVE,
                mybir.EngineType.Activation,
            )
            with tc.For_i(0, reps, 1, hint_engines=hints) as _i:
                body(_i)

    nc.compile()
    _CACHE[key] = nc
    return nc


def _make_in_maps(inputs):
    x = np.asarray(inputs["x"], np.float32)
    Wq = np.asarray(inputs["Wq"], np.float32)
    bq = np.asarray(inputs["bq"], np.float32)
    Wk = np.asarray(inputs["Wk"], np.float32)
    bk = np.asarray(inputs["bk"], np.float32)
    Wv = np.asarray(inputs["Wv"], np.float32)
    bv = np.asarray(inputs["bv"], np.float32)
    Wo = np.asarray(inputs["Wo"], np.float32)

    try:
        import ml_dtypes

        bf = ml_dtypes.bfloat16

        def to_bf16(a):
            return np.ascontiguousarray(a.astype(bf))
    except ImportError:
        def to_bf16(a):
            # round-to-nearest-even fp32 -> bf16, stored as uint16
            u = np.ascontiguousarray(a, np.float32).view(np.uint32)
            rounded = (u + 0x7FFF + ((u >> 16) & 1)) >> 16
            return np.ascontiguousarray(rounded.astype(np.uint16))

    tri = np.triu(np.ones((128, 128), np.float32))
    in_maps = []
    for c in range(8):
        b, tp = c // 4, c % 4
        k0, k1 = 2 * tp, 2 * tp + 1
        qorder = np.concatenate(
            [
                np.r_[(4 * k + j) * D:(4 * k + j + 1) * D]
                for j in range(4)
                for k in (k0, k1)
            ]
        )
        kvorder = np.r_[k0 * D:(k0 + 1) * D, k1 * D:(k1 + 1) * D]
        bias = np.zeros((128, 6), np.float32)
        bias[:, 0:4] = bq[qorder].reshape(4, 128).T
        bias[:, 4] = bk[kvorder][0:128]
        bias[:, 5] = bv[kvorder][0:128]
        in_maps.append(
            {
                "xt": to_bf16(x[b].T),
                "wq": to_bf16(Wq[:, qorder]),
                "wk": to_bf16(Wk[:, kvorder]),
                "wv": to_bf16(Wv[:, kvorder]),
                "wo": to_bf16(Wo[qorder, :]),
                "bias": bias,
                "tri": to_bf16(tri),
            }
        )
    return in_maps


def _gather(results, bo):
    out = np.zeros((2, T, C), np.float32)
    for c in range(8):
        out[c // 4] += results[c]["opt"].astype(np.float32).T
    out += bo.astype(np.float32)
    return out


def kernel(**inputs):
    _ensure_path()
    from concourse.bass_utils import run_bass_kernel_spmd

    nc = _build(reps=1)
    in_maps = _make_in_maps(inputs)
    res = run_bass_kernel_spmd(nc, in_maps, list(range(8)))
    return _gather(res.results, np.asarray(inputs["bo"], np.float32))


def run_timed(inputs, reps, n_calls=3, sections=("p1", "attn", "oproj")):
    """Wall-clock the SPMD call at a given in-kernel rep count; returns
    (best_wall_seconds, outputs). Kernel time per rep is isolated by
    differencing two rep counts (data transfer is identical)."""
    import time

    _ensure_path()
    from concourse.bass_utils import run_bass_kernel_spmd

    nc = _build(reps=reps, sections=sections)
    in_maps = _make_in_maps(inputs)
    best = None
    res = None
    for _ in range(n_calls):
        t0 = time.time()
        res = run_bass_kernel_spmd(nc, in_maps, list(range(8)))
        dtm = time.time() - t0
        best = dtm if best is None else min(best, dtm)
    return best, _gather(res.results, np.asarray(inputs["bo"], np.float32))
